# revision 4
# baseline (speedup 1.0000x reference)
"""Trainium2 Bass kernel for nn_ASA_37692632989803 (topk_masking).

Reference computation (n=32, t=8, c=128, h=w=32):
  1. per-(n,t,c): avg/max pool over (h,w); map = (0.5+alpha)*avg + (0.5+beta)*max
  2. FC along t (W_fc), top-k masks (kc=64 over c, kt=4 over t) with batch-union
     (requires cross-device AllReduce-max of the one-hot masks)
  3. im/sub masked pools over (t,c), 3x3 conv (2ch->1) + sigmoid per branch
  4. out = im_map*imf + sub_map*subf

Sharding: data-parallel over batch n (4 images per core, 8 cores), with an
AllReduce-max collective for the [t,c] union masks.
"""
import sys
import os
import numpy as np

for _p in ("/opt/trn_rl_repo",):
    if _p not in sys.path:
        sys.path.insert(0, _p)

N, T, C, H, Wd = 32, 8, 128, 32, 32
HW = H * Wd
NCORES = 8
NLOC = N // NCORES          # 4 images per core
NT = NLOC * T               # 32 (n,t) tiles per core
KC, KT = 64, 4              # top-k sizes (LAM = 0.5)

_CACHE = {}


def _emit(ctx, tc):
    import concourse.bass as bass
    from concourse import mybir
    from concourse.masks import make_identity

    nc = tc.nc
    f32 = mybir.dt.float32
    A = mybir.AluOpType
    AF = mybir.ActivationFunctionType
    AX = mybir.AxisListType
    sync, vec, act, gp, pe = nc.sync, nc.vector, nc.scalar, nc.gpsimd, nc.tensor

    x_d = nc.dram_tensor("x", [NLOC, T, C, HW], f32, kind="ExternalInput").ap()
    al_d = nc.dram_tensor("alpha", [1], f32, kind="ExternalInput").ap()
    be_d = nc.dram_tensor("beta", [1], f32, kind="ExternalInput").ap()
    wfc_d = nc.dram_tensor("wfc", [T, T], f32, kind="ExternalInput").ap()
    w1_d = nc.dram_tensor("w1", [2, 3, 3], f32, kind="ExternalInput").ap()
    w2_d = nc.dram_tensor("w2", [2, 3, 3], f32, kind="ExternalInput").ap()
    out_d = nc.dram_tensor("out", [NLOC, T, C, HW], f32, kind="ExternalOutput").ap()

    singles = ctx.enter_context(tc.tile_pool(name="singles", bufs=1))
    xpool = ctx.enter_context(tc.tile_pool(name="x", bufs=NT))
    accpool = ctx.enter_context(tc.tile_pool(name="acc", bufs=2))
    outpool = ctx.enter_context(tc.tile_pool(name="outp", bufs=3))
    scratch = ctx.enter_context(tc.tile_pool(name="scratch", bufs=1))
    mpool = ctx.enter_context(tc.tile_pool(name="mpool", bufs=2))
    dram = ctx.enter_context(tc.tile_pool(name="dram", bufs=1, space="DRAM"))
    ps_small = ctx.enter_context(tc.tile_pool(name="ps_small", bufs=2, space="PSUM"))
    ps_sums = ctx.enter_context(tc.tile_pool(name="ps_sums", bufs=1, space="PSUM"))
    ps_gain = ctx.enter_context(tc.tile_pool(name="ps_gain", bufs=2, space="PSUM"))

    ident = singles.tile([128, 128], f32, tag="ident")
    make_identity(nc, ident)

    # ---------------- phase A: load x, pooled stats ----------------
    stats_sum = singles.tile([C, NT], f32, tag="ssum")
    stats_max = singles.tile([C, NT], f32, tag="smax")
    ascratch = singles.tile([C, HW], f32, tag="ascratch")
    xt = []
    for k in range(NT):
        n, t = divmod(k, T)
        xtile = xpool.tile([C, HW], f32, tag="xt")
        xt.append(xtile)
        sync.dma_start(xtile, x_d[n, t])
        vec.tensor_reduce(stats_max[:, k:k + 1], xtile, axis=AX.X, op=A.max)
        act.activation(ascratch, xtile, AF.Copy,
                       accum_out=stats_sum[:, k:k + 1])

    # ma = (0.5+alpha)/HW * sum + (0.5+beta) * max   (layout [c, (n,t)])
    ab_s = singles.tile([1, 2], f32, tag="ab_s")
    sync.dma_start(ab_s[0:1, 0:1], al_d.unsqueeze(0))
    sync.dma_start(ab_s[0:1, 1:2], be_d.unsqueeze(0))
    ab_b = singles.tile([C, 2], f32, tag="ab_b")
    gp.partition_broadcast(ab_b, ab_s)
    s12 = singles.tile([C, 2], f32, tag="s12")
    act.activation(s12[:, 0:1], ab_b[:, 0:1], AF.Copy, scale=1.0 / HW, bias=0.5 / HW)
    act.activation(s12[:, 1:2], ab_b[:, 1:2], AF.Copy, scale=1.0, bias=0.5)
    ma = singles.tile([C, NT], f32, tag="ma")
    vec.tensor_scalar(ma, stats_sum, s12[:, 0:1], None, op0=A.mult)
    vec.scalar_tensor_tensor(ma, stats_max, s12[:, 1:2], ma, A.mult, A.add)

    # ---------------- FC along t:  Y[(n,s), c] ----------------
    maT_ps = ps_small.tile([NT, C], f32, tag="psm")
    pe.transpose(maT_ps, ma, ident)
    maT = singles.tile([NT, C], f32, tag="maT")
    vec.tensor_copy(maT, maT_ps)
    wt = singles.tile([T, T], f32, tag="wt")
    sync.dma_start(wt, wfc_d.transpose([1, 0]))          # wt[t, s] = W_fc[s, t]
    Lb = singles.tile([NT, NT], f32, tag="Lb")
    vec.memset(Lb, 0.0)
    for i in range(NLOC):
        sync.dma_start(Lb[8 * i:8 * i + 8, 8 * i:8 * i + 8], wt)
    Y_ps = ps_small.tile([NT, C], f32, tag="psm")
    pe.matmul(Y_ps, Lb, maT, start=True, stop=True)
    Ysb = singles.tile([NT, C], f32, tag="Ysb")
    vec.tensor_copy(Ysb, Y_ps)

    # ---------------- top-k masks via pairwise ranks ----------------
    # rank_c[(n,s), c] = #{c' : Y[ns,c'] > Y[ns,c]};  mask_c = rank < KC
    rank_c = singles.tile([NT, C], f32, tag="rank_c")
    CH = 16
    for b in range(C // CH):
        cmp = scratch.tile([NT, CH * C], f32, tag="cmp")
        cmp3 = cmp.rearrange("p (a b) -> p a b", a=CH)
        in0 = Ysb.unsqueeze(1).broadcast_to([NT, CH, C])
        in1 = Ysb[:, b * CH:(b + 1) * CH].unsqueeze(2).broadcast_to([NT, CH, C])
        vec.tensor_tensor(cmp3, in0, in1, op=A.is_gt)
        vec.tensor_reduce(rank_c[:, b * CH:(b + 1) * CH], cmp3, axis=AX.X, op=A.add)
    mask_c = singles.tile([NT, C], f32, tag="mask_c")
    vec.tensor_scalar(mask_c, rank_c, KC - 0.5, None, op0=A.is_lt)
    mcT_ps = ps_small.tile([C, NT], f32, tag="psm")
    pe.transpose(mcT_ps, mask_c, ident[0:NT, 0:NT])
    mcT = singles.tile([C, NT], f32, tag="mcT")
    vec.tensor_copy(mcT, mcT_ps)
    masks_l = singles.tile([C, 16], f32, tag="masks_l")
    vec.tensor_tensor(masks_l[:, 0:8], mcT[:, 0:8], mcT[:, 8:16], op=A.max)
    vec.tensor_tensor(masks_l[:, 0:8], masks_l[:, 0:8], mcT[:, 16:24], op=A.max)
    vec.tensor_tensor(masks_l[:, 0:8], masks_l[:, 0:8], mcT[:, 24:32], op=A.max)

    # rank_t over time (per (n,c)):  YT [c, (n,s)]
    YT_ps = ps_small.tile([C, NT], f32, tag="psm")
    pe.transpose(YT_ps, Ysb, ident[0:NT, 0:NT])
    YT = singles.tile([C, NT], f32, tag="YT")
    vec.tensor_copy(YT, YT_ps)
    cmp_t = singles.tile([C, NLOC * T * T], f32, tag="cmp_t")
    YT3 = YT.rearrange("p (n s) -> p n s", n=NLOC)
    cin0 = YT3.unsqueeze(2).broadcast_to([C, NLOC, T, T])
    cin1 = YT3.unsqueeze(3).broadcast_to([C, NLOC, T, T])
    cmp_t4 = cmp_t.rearrange("p (n s sp) -> p n s sp", n=NLOC, s=T)
    vec.tensor_tensor(cmp_t4, cin0, cin1, op=A.is_gt)
    rank_t = singles.tile([C, NT], f32, tag="rank_t")
    rank_t3 = rank_t.rearrange("p (n s) -> p n s", n=NLOC)
    vec.tensor_reduce(rank_t3, cmp_t4, axis=AX.X, op=A.add)
    mask_t = singles.tile([C, NT], f32, tag="mask_t")
    vec.tensor_scalar(mask_t, rank_t, KT - 0.5, None, op0=A.is_lt)
    vec.tensor_tensor(masks_l[:, 8:16], mask_t[:, 0:8], mask_t[:, 8:16], op=A.max)
    vec.tensor_tensor(masks_l[:, 8:16], masks_l[:, 8:16], mask_t[:, 16:24], op=A.max)
    vec.tensor_tensor(masks_l[:, 8:16], masks_l[:, 8:16], mask_t[:, 24:32], op=A.max)

    # ---------------- AllReduce-max union across cores ----------------
    cc_in = dram.tile([C, 16], f32, tag="cc_in")
    cc_out = dram.tile([C, 16], f32, tag="cc_out")
    sync.dma_start(cc_in, masks_l)
    gp.collective_compute(
        "AllReduce", A.max,
        replica_groups=[list(range(NCORES))],
        ins=[cc_in.opt()], outs=[cc_out.opt()],
    )
    masks_g = singles.tile([C, 16], f32, tag="masks_g")
    sync.dma_start(masks_g, cc_out)

    # im/sub columns, interleaved: cols2[:, 2t] = im, cols2[:, 2t+1] = sub
    cols2 = singles.tile([C, 16], f32, tag="cols2")
    cols2_3 = cols2.rearrange("p (t two) -> p t two", two=2)
    tmp8 = singles.tile([C, T], f32, tag="tmp8")
    vec.tensor_tensor(tmp8, masks_g[:, 0:8], masks_g[:, 8:16], op=A.add)
    vec.tensor_scalar(cols2_3[:, :, 0], tmp8, 1.0, None, op0=A.is_equal)
    act.activation(cols2_3[:, :, 1], cols2_3[:, :, 0], AF.Copy, scale=-1.0, bias=1.0)

    # transposed [im;sub] pairs for the gain matmul, one [2, C] pair per t
    colsT_ps = ps_small.tile([16, C], f32, tag="psm")
    pe.transpose(colsT_ps, cols2, ident)
    colsT = singles.tile([16, C], f32, tag="colsT")
    vec.tensor_copy(colsT, colsT_ps)
    pairs = singles.tile([2, T * C], f32, tag="pairs")
    for t in range(T):
        sync.dma_start(pairs[:, t * C:(t + 1) * C], colsT[2 * t:2 * t + 2, :])

    # ---------------- conv setup: banded L matrices ----------------
    # wsel[(ch,py), cv*9 + dy*3 + dx] = W_conv[cv][ch, dy, dx]; ch0 rows scaled 1/512
    wsel = singles.tile([68, 18], f32, tag="wsel")
    w1r = w1_d.rearrange("a b c -> a (b c)")
    w2r = w2_d.rearrange("a b c -> a (b c)")
    sync.dma_start(wsel[:, 0:9], w1r.unsqueeze(1).broadcast_to([2, 34, 9]))
    sync.dma_start(wsel[:, 9:18], w2r.unsqueeze(1).broadcast_to([2, 34, 9]))
    vec.tensor_scalar(wsel[0:34, :], wsel[0:34, :], 1.0 / 512.0, None, op0=A.mult)
    # Band_dy [68, 32]: 1 at py - y == dy for both 34-row channel blocks
    bands = singles.tile([68, 96], f32, tag="bands")
    gp.memset(bands, 0.0)
    for dy in range(3):
        bsl = bands[:, 32 * dy:32 * dy + 32]
        gp.affine_select(bsl, bsl, pattern=[[-1, 32]], compare_op=A.not_equal,
                         fill=1.0, base=-dy, channel_multiplier=1)
        gp.affine_select(bsl, bsl, pattern=[[-1, 32]], compare_op=A.not_equal,
                         fill=1.0, base=-(34 + dy), channel_multiplier=1)
    # L[cv,dx] [68, 32] = sum_dy Band_dy * wsel[:, cv*9+dy*3+dx]
    L_all = singles.tile([68, 6 * 32], f32, tag="L_all")
    for cv in range(2):
        for dx in range(3):
            L = L_all[:, (cv * 3 + dx) * 32:(cv * 3 + dx) * 32 + 32]
            w0 = wsel[:, cv * 9 + dx:cv * 9 + dx + 1]
            w1c = wsel[:, cv * 9 + 3 + dx:cv * 9 + 3 + dx + 1]
            w2c = wsel[:, cv * 9 + 6 + dx:cv * 9 + 6 + dx + 1]
            vec.tensor_scalar(L, bands[:, 0:32], w0, None, op0=A.mult)
            vec.scalar_tensor_tensor(L, bands[:, 32:64], w1c, L, A.mult, A.add)
            vec.scalar_tensor_tensor(L, bands[:, 64:96], w2c, L, A.mult, A.add)

    # ---------------- phase B + conv + phase C, pipelined over n ----------------
    for n in range(NLOC):
        acc_im = accpool.tile([C, HW], f32, tag="accim")
        acc_sub = accpool.tile([C, HW], f32, tag="accsub")
        ps_sum = ps_sums.tile([2, HW], f32, tag="pssum")
        for t in range(T):
            xtile = xt[n * T + t]
            im_col = cols2[:, 2 * t:2 * t + 1]
            sub_col = cols2[:, 2 * t + 1:2 * t + 2]
            if t == 0:
                vec.tensor_scalar(acc_im, xtile, im_col, None, op0=A.mult)
                vec.tensor_scalar(acc_sub, xtile, sub_col, None, op0=A.mult)
            else:
                vec.scalar_tensor_tensor(acc_im, xtile, im_col, acc_im, A.mult, A.max)
                vec.scalar_tensor_tensor(acc_sub, xtile, sub_col, acc_sub, A.mult, A.max)
            for h2 in range(2):
                pe.matmul(ps_sum[:, 512 * h2:512 * h2 + 512],
                          cols2[:, 2 * t:2 * t + 2],
                          xtile[:, 512 * h2:512 * h2 + 512],
                          start=(t == 0), stop=(t == T - 1))
        sums_sb = mpool.tile([2, HW], f32, tag="sums")
        act.activation(sums_sb, ps_sum, AF.Copy)
        # partition-max of acc_{im,sub} via PE transposes
        hw_im = mpool.tile([C, 8], f32, tag="hwim")
        hw_sub = mpool.tile([C, 8], f32, tag="hwsub")
        for b in range(8):
            tp = ps_small.tile([128, 128], f32, tag="psm")
            pe.transpose(tp, acc_im[:, 128 * b:128 * b + 128], ident)
            vec.tensor_reduce(hw_im[:, b:b + 1], tp, axis=AX.X, op=A.max)
            tp2 = ps_small.tile([128, 128], f32, tag="psm")
            pe.transpose(tp2, acc_sub[:, 128 * b:128 * b + 128], ident)
            vec.tensor_reduce(hw_sub[:, b:b + 1], tp2, axis=AX.X, op=A.max)
        hwT_im_ps = ps_small.tile([8, C], f32, tag="psm")
        pe.transpose(hwT_im_ps, hw_im, ident)
        hwT_im = mpool.tile([8, C], f32, tag="hwTim")
        vec.tensor_copy(hwT_im, hwT_im_ps)
        hwT_sub_ps = ps_small.tile([8, C], f32, tag="psm")
        pe.transpose(hwT_sub_ps, hw_sub, ident)
        hwT_sub = mpool.tile([8, C], f32, tag="hwTsub")
        vec.tensor_copy(hwT_sub, hwT_sub_ps)

        maps2 = mpool.tile([2, HW], f32, tag="maps2")
        for cv, (sumrow, hwT) in enumerate(
                [(sums_sb[0:1, :], hwT_im), (sums_sb[1:2, :], hwT_sub)]):
            pad = mpool.tile([68, 34], f32, tag="pad")
            gp.memset(pad, 0.0)
            sync.dma_start(pad[1:33, 1:33], sumrow.rearrange("p (a b) -> p a b", a=32))
            sync.dma_start(pad[35:67, 1:33], hwT.rearrange("p (a b) -> p a b", a=4))
            conv_ps = ps_small.tile([32, 32], f32, tag="psm")
            for dx in range(3):
                pe.matmul(conv_ps, L_all[:, (cv * 3 + dx) * 32:(cv * 3 + dx) * 32 + 32],
                          pad[:, dx:dx + 32], start=(dx == 0), stop=(dx == 2))
            sig = mpool.tile([32, 32], f32, tag="sig")
            act.activation(sig, conv_ps, AF.Sigmoid)
            sync.dma_start(maps2[cv:cv + 1, :], sig)

        # phase C: gain = pairs[t].T @ maps2  (outer product), out = x * gain
        for t in range(T):
            g_ps = ps_gain.tile([C, HW], f32, tag="gain")
            for h2 in range(2):
                pe.matmul(g_ps[:, 512 * h2:512 * h2 + 512],
                          pairs[:, t * C:(t + 1) * C],
                          maps2[:, 512 * h2:512 * h2 + 512],
                          start=True, stop=True)
            o = outpool.tile([C, HW], f32, tag="o")
            vec.tensor_tensor(o, xt[n * T + t], g_ps, op=A.mult)
            sync.dma_start(out_d[n, t], o)


def build_nc():
    from contextlib import ExitStack
    import concourse.tile as tile
    from concourse import bacc

    nc = bacc.Bacc("TRN2", debug=False, enable_asserts=False,
                   target_bir_lowering=False, num_devices=NCORES)
    with tile.TileContext(nc) as tc, ExitStack() as ctx:
        _emit(ctx, tc)
    nc.compile()
    return nc


def _get_nc():
    if "nc" not in _CACHE:
        _CACHE["nc"] = build_nc()
    return _CACHE["nc"]


def make_in_maps(inputs):
    x = np.ascontiguousarray(np.asarray(inputs["x"], dtype=np.float32))
    alpha = np.asarray(inputs["alpha"], np.float32).reshape(1)
    beta = np.asarray(inputs["beta"], np.float32).reshape(1)
    wfc = np.ascontiguousarray(np.asarray(inputs["W_fc"], np.float32))
    w1 = np.ascontiguousarray(np.asarray(inputs["W_conv1"], np.float32).reshape(2, 3, 3))
    w2 = np.ascontiguousarray(np.asarray(inputs["W_conv2"], np.float32).reshape(2, 3, 3))
    in_maps = []
    for i in range(NCORES):
        shard = np.ascontiguousarray(
            x[i * NLOC:(i + 1) * NLOC].reshape(NLOC, T, C, HW))
        in_maps.append({"x": shard, "alpha": alpha, "beta": beta,
                        "wfc": wfc, "w1": w1, "w2": w2})
    return in_maps


def gather_out(results):
    return np.concatenate(
        [r["out"].reshape(NLOC, T, C, H, Wd) for r in results], axis=0)


def kernel(**inputs):
    from concourse.bass_utils import run_bass_kernel_spmd
    nc = _get_nc()
    res = run_bass_kernel_spmd(nc, make_in_maps(inputs),
                               core_ids=list(range(NCORES)))
    return gather_out(res.results)


if __name__ == "__main__":
    nc = build_nc()
    print("build + compile OK")
